# revision 19
# baseline (speedup 1.0000x reference)
"""Trainium2 Bass kernel for nn_MultiHeadAttention_53815940219243.

Reference computation (single-head attention with full 1024-dim contraction):
    q = x @ Wq + bq; k = x @ Wk + bk; v = x @ Wv + bv        # [4096, 1024]
    scores = softmax(q @ k.T, axis=-1) / sqrt(64)            # [4096, 4096]
    z = scores @ v                                           # [4096, 1024]
    out = z @ Wo + bo                                        # [4096, 64]

Sharding: sequence dim of Q/scores/output split across 8 cores (512 rows each).
K and V are computed sharded (each core its own 512 rows) and exchanged with
two AllGather collectives overlapped with the other projections.

Dataflow runs in "transposed score space" so no on-device transposes are
needed (see ST/UT below). High-precision matmuls use hi/lo-split bf16
operands (3 passes: hh, hl, lh), giving ~2e-4 matmul error at bf16 speed;
E=exp(S) and v are single bf16 (~1e-3), the final Wo matmul is fp32.

Softmax is computed without max subtraction: scores are ~N(0, 10.7^2), so
|S| < ~60 and exp(S) stays inside fp32/bf16 range.
"""

import numpy as np

N = 4096
D = 1024
H = 64
NCORES = 8
NSH = N // NCORES  # 512 rows per core
P = 128
DT = D // P  # 8 partition tiles over the 1024 feature dim
JT = N // P  # 32 partition tiles over the full sequence
JSH = NSH // P  # 4 sequence tiles per core shard
IT = NSH // P  # 4 output row tiles per core

_CACHE = {}

# S-matmul mode: False = hi/lo-split bf16 (3 passes), True = single-pass fp16
S_FP16 = True
# projection mode: True = single-pass fp16 projections
PROJ_FP16 = True


def _build(upto="O", rep_a=1, rep_ag=1, rep_s=1, rep_u=1, rep_o=1, s_fp16=False, proj_fp16=False):
    import concourse.mybir as mybir
    import concourse.tile as tile
    from concourse import bacc
    from contextlib import ExitStack

    stages = {"A0": 0, "A": 0, "S": 1, "U": 2, "O": 3}
    lvl = stages[upto]
    no_ag = upto == "A0"

    f32 = mybir.dt.float32
    bf16 = mybir.dt.bfloat16
    fp16 = mybir.dt.float16
    sdt = fp16 if s_fp16 else bf16

    nc = bacc.Bacc("TRN2", target_bir_lowering=False, num_devices=NCORES)

    # ---- kernel I/O (per core) ----
    pdt = fp16 if proj_fp16 else bf16
    xh = nc.dram_tensor("xh", [D, NSH], pdt, kind="ExternalInput")
    xl = nc.dram_tensor("xl", [D, NSH], pdt, kind="ExternalInput")
    # weights pre-arranged on host: [t, p, dt*c] so each dout-tile load is
    # one contiguous-per-partition DMA
    wqh = nc.dram_tensor("wqh", [DT, P, D], pdt, kind="ExternalInput")
    wql = nc.dram_tensor("wql", [DT, P, D], pdt, kind="ExternalInput")
    wkh = nc.dram_tensor("wkh", [DT, P, D], pdt, kind="ExternalInput")
    wkl = nc.dram_tensor("wkl", [DT, P, D], pdt, kind="ExternalInput")
    wvh = nc.dram_tensor("wvh", [2, P, DT * 512], pdt, kind="ExternalInput")
    wvl = nc.dram_tensor("wvl", [2, P, DT * 512], pdt, kind="ExternalInput")
    wo = nc.dram_tensor("wo", [D, H], fp16, kind="ExternalInput")
    bq_r = nc.dram_tensor("bq_r", [1, D], pdt, kind="ExternalInput")
    bk_r = nc.dram_tensor("bk_r", [1, D], pdt, kind="ExternalInput")
    bv_r = nc.dram_tensor("bv_r", [1, D], pdt, kind="ExternalInput")
    bo_r = nc.dram_tensor("bo_r", [1, H], fp16, kind="ExternalInput")
    ones_row = nc.dram_tensor("ones_row", [1, NSH], pdt, kind="ExternalInput")
    ones_col = nc.dram_tensor("ones_col", [P, 1], bf16, kind="ExternalInput")
    eighth_row = nc.dram_tensor("eighth_row", [1, P], f32, kind="ExternalInput")
    ones32_row = nc.dram_tensor("ones32_row", [1, P], fp16, kind="ExternalInput")

    out = nc.dram_tensor("out", [NSH, H], f32, kind="ExternalOutput")

    # ---- internal DRAM for the collectives ----
    # layouts are partition-major so the post-gather streaming loads read
    # 16KB/8KB contiguous per partition line
    nkt = 1 if s_fp16 else 2
    ag_in_kt = nc.dram_tensor("ag_in_kt", [P, nkt, DT, NSH], sdt)
    ag_out_kt = nc.dram_tensor("ag_out_kt", [NCORES, P, nkt, DT, NSH], sdt, addr_space="Shared")
    ag_in_v = nc.dram_tensor("ag_in_v", [P, JSH, D], bf16)
    ag_out_v = nc.dram_tensor("ag_out_v", [NCORES, P, JSH, D], bf16, addr_space="Shared")

    with tile.TileContext(nc) as tc, ExitStack() as ctx:
        persist = ctx.enter_context(tc.tile_pool(name="persist", bufs=1))
        small = ctx.enter_context(tc.tile_pool(name="small", bufs=1))

        qth_sb = persist.tile([P, DT, NSH], sdt, tag="qth")
        qtl_sb = persist.tile([P, DT, NSH], sdt, tag="qtl")
        et_sb = persist.tile([P, JT, NSH], bf16, tag="et")        # 32KB/part
        vf_sb = persist.tile([P, NCORES, JSH, D], bf16, tag="vf")  # 64KB/part
        zt_sb = persist.tile([P, DT, NSH], fp16, tag="zt")        # 8KB/part
        sbc_sb = persist.tile([P, NSH], f32, tag="sbc")

        ones_row_sb = small.tile([1, NSH], pdt, tag="onesr")
        ones_col_sb = small.tile([P, 1], bf16, tag="onesc")
        eighth_sb = small.tile([1, P], f32, tag="eighth")
        ones32_sb = small.tile([1, P], fp16, tag="ones32")
        bq_sb = small.tile([1, D], pdt, tag="bq")
        bk_sb = small.tile([1, D], pdt, tag="bk")
        bv_sb = small.tile([1, D], pdt, tag="bv")
        bo_sb = small.tile([1, H], fp16, tag="bo")
        wo_sb = small.tile([P, DT, H], fp16, tag="wo")
        den_row = small.tile([1, NSH], f32, tag="denr")

        nc.sync.dma_start(out=ones_row_sb[:], in_=ones_row[:, :])
        nc.sync.dma_start(out=ones_col_sb[:], in_=ones_col[:, :])
        nc.sync.dma_start(out=eighth_sb[:], in_=eighth_row[:, :])
        nc.sync.dma_start(out=ones32_sb[:], in_=ones32_row[:, :])
        nc.sync.dma_start(out=bq_sb[:], in_=bq_r[:, :])
        nc.sync.dma_start(out=bk_sb[:], in_=bk_r[:, :])
        nc.sync.dma_start(out=bv_sb[:], in_=bv_r[:, :])
        nc.sync.dma_start(out=bo_sb[:], in_=bo_r[:, :])
        nc.sync.dma_start(out=wo_sb[:], in_=wo[:, :].rearrange("(t p) h -> p t h", p=P))

        # ---------------- phase A: projections ----------------
        with (
            tc.tile_pool(name="pa_x", bufs=1) as pax,
            tc.tile_pool(name="pa_sbuf", bufs=3) as pa,
            tc.tile_pool(name="pa_w", bufs=2) as paw,
            tc.tile_pool(name="pa_psum", bufs=3, space="PSUM") as pap,
        ):
            xh_sb = pax.tile([P, DT, NSH], pdt, tag="xh")
            xl_sb = pax.tile([P, DT, NSH], pdt, tag="xl")
            nc.sync.dma_start(out=xh_sb[:], in_=xh[:, :].rearrange("(t p) i -> p t i", p=P))
            nc.sync.dma_start(out=xl_sb[:], in_=xl[:, :].rearrange("(t p) i -> p t i", p=P))

            def proj_qk(w_h, w_l, b_sb, t):
                """One dout-tile of a hi/lo-split projection W^T @ xT + b."""
                wt_h = paw.tile([P, DT, P], pdt, tag="wqkh")
                nc.sync.dma_start(out=wt_h[:], in_=w_h[t].rearrange("p (dt c) -> p dt c", c=P))
                if not proj_fp16:
                    wt_l = paw.tile([P, DT, P], pdt, tag="wqkl")
                    nc.sync.dma_start(out=wt_l[:], in_=w_l[t].rearrange("p (dt c) -> p dt c", c=P))
                ps = pap.tile([P, NSH], mybir.dt.float32, tag="pa")
                for dt_i in range(DT):
                    nc.tensor.matmul(ps[:], wt_h[:, dt_i, :], xh_sb[:, dt_i, :],
                                     start=(dt_i == 0), stop=False)
                    if not proj_fp16:
                        nc.tensor.matmul(ps[:], wt_h[:, dt_i, :], xl_sb[:, dt_i, :],
                                         start=False, stop=False)
                        nc.tensor.matmul(ps[:], wt_l[:, dt_i, :], xh_sb[:, dt_i, :],
                                         start=False, stop=False)
                nc.tensor.matmul(ps[:], b_sb[0:1, t * P : (t + 1) * P],
                                 ones_row_sb[0:1, :], start=False, stop=True)
                return ps

            # K^T shard -> split hi/lo (or single fp16) -> ag_in_kt
            for _ra in range(rep_a):
              for t in range(DT):
                ps = proj_qk(wkh, wkl, bk_sb, t)
                kt_h = pa.tile([P, NSH], sdt, tag="kth")
                nc.vector.tensor_copy(out=kt_h[:], in_=ps[:])
                nc.sync.dma_start(out=ag_in_kt[:, 0, t, :], in_=kt_h[:])
                if not s_fp16:
                    kt_l = pa.tile([P, NSH], sdt, tag="ktl")
                    nc.vector.tensor_sub(out=kt_l[:], in0=ps[:], in1=kt_h[:])
                    nc.sync.dma_start(out=ag_in_kt[:, 1, t, :], in_=kt_l[:])

            if not no_ag:
                for _rg in range(rep_ag):
                    nc.gpsimd.collective_compute(
                        "AllGather", mybir.AluOpType.bypass,
                        replica_groups=[list(range(NCORES))],
                        ins=[ag_in_kt[:, :, :, :].opt()],
                        outs=[ag_out_kt[:, :, :, :, :].opt()],
                    )

            # v shard (natural layout, single bf16) -> ag_in_v
            for _ra in range(rep_a):
              for b in range(2):
                wv_h = paw.tile([P, DT, 512], pdt, tag="wvh")
                nc.sync.dma_start(out=wv_h[:], in_=wvh[b].rearrange("p (dt c) -> p dt c", c=512))
                if not proj_fp16:
                    wv_l = paw.tile([P, DT, 512], pdt, tag="wvl")
                    nc.sync.dma_start(out=wv_l[:], in_=wvl[b].rearrange("p (dt c) -> p dt c", c=512))
                for j in range(JSH):
                    ps = pap.tile([P, 512], mybir.dt.float32, tag="pa")
                    for dt_i in range(DT):
                        nc.tensor.matmul(ps[:], xh_sb[:, dt_i, j * P : (j + 1) * P],
                                         wv_h[:, dt_i, :], start=(dt_i == 0), stop=False)
                        if not proj_fp16:
                            nc.tensor.matmul(ps[:], xh_sb[:, dt_i, j * P : (j + 1) * P],
                                             wv_l[:, dt_i, :], start=False, stop=False)
                            nc.tensor.matmul(ps[:], xl_sb[:, dt_i, j * P : (j + 1) * P],
                                             wv_h[:, dt_i, :], start=False, stop=False)
                    nc.tensor.matmul(ps[:], ones_row_sb[0:1, 0:P],
                                     bv_sb[0:1, b * 512 : (b + 1) * 512],
                                     start=False, stop=True)
                    v_t = pa.tile([P, 512], bf16, tag="vsh")
                    nc.vector.tensor_copy(out=v_t[:], in_=ps[:])
                    nc.sync.dma_start(
                        out=ag_in_v[:, j, b * 512 : (b + 1) * 512], in_=v_t[:]
                    )

            if not no_ag:
                for _rg in range(rep_ag):
                    nc.gpsimd.collective_compute(
                        "AllGather", mybir.AluOpType.bypass,
                        replica_groups=[list(range(NCORES))],
                        ins=[ag_in_v[:, :, :].opt()],
                        outs=[ag_out_v[:, :, :, :].opt()],
                    )

            # Q^T (resident, split hi/lo or single fp16)
            for _ra in range(rep_a):
              for t in range(DT):
                ps = proj_qk(wqh, wql, bq_sb, t)
                nc.vector.tensor_copy(out=qth_sb[:, t, :], in_=ps[:])
                if not s_fp16:
                    nc.vector.tensor_sub(out=qtl_sb[:, t, :], in0=ps[:], in1=qth_sb[:, t, :])

        if lvl >= 1:
            # full v load (overlaps the S phase below)
            nc.sync.dma_start(
                out=vf_sb[:],
                in_=ag_out_v[:, :, :, :].rearrange("r p q d -> p r q d"),
            )

            # ------------- phase S: scores + exp + denominator -------------
            with (
                tc.tile_pool(name="ps_kt", bufs=3) as pskt,
                tc.tile_pool(name="ps_psum", bufs=4, space="PSUM") as psp,
                tc.tile_pool(name="ps_den", bufs=1, space="PSUM") as psd,
            ):
                den_ps = psd.tile([1, NSH], mybir.dt.float32, tag="den")
                for _rs in range(rep_s):
                  for r in range(NCORES):
                    kt_r = pskt.tile([P, nkt, DT, NSH], sdt, tag="ktr")
                    nc.sync.dma_start(out=kt_r[:], in_=ag_out_kt[r, :, :, :, :])
                    for jj in range(JSH):
                        jt = r * JSH + jj
                        ps = psp.tile([P, NSH], mybir.dt.float32, tag="st")
                        jsl = slice(jj * P, (jj + 1) * P)
                        for dt_i in range(DT):
                            last = dt_i == DT - 1
                            if s_fp16:
                                nc.tensor.matmul(ps[:], kt_r[:, 0, dt_i, jsl],
                                                 qth_sb[:, dt_i, :],
                                                 start=(dt_i == 0), stop=last)
                            else:
                                nc.tensor.matmul(ps[:], kt_r[:, 0, dt_i, jsl], qth_sb[:, dt_i, :],
                                                 start=(dt_i == 0), stop=False)
                                nc.tensor.matmul(ps[:], kt_r[:, 0, dt_i, jsl], qtl_sb[:, dt_i, :],
                                                 start=False, stop=False)
                                nc.tensor.matmul(ps[:], kt_r[:, 1, dt_i, jsl], qth_sb[:, dt_i, :],
                                                 start=False, stop=last)
                        nc.scalar.activation(out=et_sb[:, jt, :], in_=ps[:],
                                             func=mybir.ActivationFunctionType.Exp)
                        if jt > 0:
                            # one-tile lag: sum exp(S) of the previous tile so
                            # the PE never waits on the current tile's exp
                            nc.tensor.matmul(den_ps[:], ones_col_sb[:, 0:1],
                                             et_sb[:, jt - 1, :],
                                             start=(jt == 1), stop=False)
                  nc.tensor.matmul(den_ps[:], ones_col_sb[:, 0:1],
                                   et_sb[:, JT - 1, :], start=False, stop=True)

                # s = 1/(8*den) broadcast to 128 partitions (fp32 matmul)
                nc.vector.reciprocal(out=den_row[:], in_=den_ps[:])
                bc_ps = psd.tile([P, NSH], mybir.dt.float32, tag="bc")
                nc.tensor.matmul(bc_ps[:], eighth_sb[0:1, :], den_row[0:1, :],
                                 start=True, stop=True)
                nc.vector.tensor_copy(out=sbc_sb[:], in_=bc_ps[:])

        if lvl >= 2:
            # ---------------- phase U: z^T ----------------
            with tc.tile_pool(name="pu_psum", bufs=2, space="PSUM") as pup:
              for _ru in range(rep_u):
                for dt_i in range(DT):
                    ps = pup.tile([P, NSH], mybir.dt.float32, tag="ut")
                    dsl = slice(dt_i * P, (dt_i + 1) * P)
                    for jt in range(JT):
                        nc.tensor.matmul(ps[:], vf_sb[:, jt // JSH, jt % JSH, dsl],
                                         et_sb[:, jt, :],
                                         start=(jt == 0), stop=(jt == JT - 1))
                    nc.vector.tensor_mul(out=zt_sb[:, dt_i, :], in0=ps[:], in1=sbc_sb[:])

        if lvl >= 3:
            # ------------- phase O: out = z @ Wo + bo (fp32) -------------
            with (
                tc.tile_pool(name="po_sbuf", bufs=2) as po,
                tc.tile_pool(name="po_psum", bufs=2, space="PSUM") as pop,
            ):
              for _ro in range(rep_o):
                for it in range(IT):
                    ps = pop.tile([P, H], mybir.dt.float32, tag="o")
                    isl = slice(it * P, (it + 1) * P)
                    for dt_i in range(DT):
                        nc.tensor.matmul(ps[:], zt_sb[:, dt_i, isl], wo_sb[:, dt_i, :],
                                         start=(dt_i == 0), stop=False)
                    nc.tensor.matmul(ps[:], ones32_sb[0:1, :], bo_sb[0:1, :],
                                     start=False, stop=True)
                    o_t = po.tile([P, H], f32, tag="osb")
                    nc.vector.tensor_copy(out=o_t[:], in_=ps[:])
                    nc.sync.dma_start(out=out[isl, :], in_=o_t[:])
        else:
            with tc.tile_pool(name="dummy_o", bufs=1) as po:
                o_t = po.tile([P, H], f32, tag="osb")
                nc.vector.memset(o_t[:], 0.0)
                for it in range(IT):
                    nc.sync.dma_start(out=out[it * P : (it + 1) * P, :], in_=o_t[:])

    nc.finalize()
    return nc


def _split_bf16(a):
    import ml_dtypes
    a = np.ascontiguousarray(a, dtype=np.float32)
    hi = a.astype(ml_dtypes.bfloat16)
    lo = (a - hi.astype(np.float32)).astype(ml_dtypes.bfloat16)
    return hi, lo


def _prep_in_maps(x, Wq, bq, Wk, bk, Wv, bv, Wo, bo, proj_fp16=False):
    import ml_dtypes
    bf = np.float16 if proj_fp16 else ml_dtypes.bfloat16
    x = np.ascontiguousarray(x, dtype=np.float32)

    def split(a):
        if proj_fp16:
            h = np.ascontiguousarray(a, np.float32).astype(np.float16)
            return h, h
        return _split_bf16(a)

    def arr_qk(W):
        h, l = split(W)
        # [din, dout] -> [t, p, dt*c]: W'[t, p, dt, c] = W[dt*128+p, t*128+c]
        def re(a):
            return np.ascontiguousarray(
                a.reshape(DT, P, DT, P).transpose(2, 1, 0, 3).reshape(DT, P, D)
            )
        return re(h), re(l)

    def arr_v(W):
        h, l = split(W)
        def re(a):
            return np.ascontiguousarray(
                a.reshape(DT, P, 2, 512).transpose(2, 1, 0, 3).reshape(2, P, DT * 512)
            )
        return re(h), re(l)

    wqh, wql = arr_qk(Wq)
    wkh, wkl = arr_qk(Wk)
    wvh, wvl = arr_v(Wv)

    shared = {
        "wqh": wqh, "wql": wql, "wkh": wkh, "wkl": wkl, "wvh": wvh, "wvl": wvl,
        "wo": np.ascontiguousarray(Wo, dtype=np.float32).astype(np.float16),
        "bq_r": np.asarray(bq, np.float32).reshape(1, D).astype(bf),
        "bk_r": np.asarray(bk, np.float32).reshape(1, D).astype(bf),
        "bv_r": np.asarray(bv, np.float32).reshape(1, D).astype(bf),
        "bo_r": np.ascontiguousarray(bo, dtype=np.float32).reshape(1, H).astype(np.float16),
        "ones_row": np.ones((1, NSH), dtype=bf),
        "ones_col": np.ones((P, 1), dtype=ml_dtypes.bfloat16),
        "eighth_row": np.full((1, P), 0.125, dtype=np.float32),
        "ones32_row": np.ones((1, P), dtype=np.float16),
    }
    in_maps = []
    for c in range(NCORES):
        xcT = np.ascontiguousarray(x[c * NSH : (c + 1) * NSH, :].T)
        xch, xcl = split(xcT)
        m = dict(shared)
        m["xh"] = xch
        m["xl"] = xcl
        in_maps.append(m)
    return in_maps


def kernel(x, Wq, bq, Wk, bk, Wv, bv, Wo, bo):
    from concourse.bass_utils import run_bass_kernel_spmd

    key = ("nc", S_FP16, PROJ_FP16)
    if key not in _CACHE:
        _CACHE[key] = _build(s_fp16=S_FP16, proj_fp16=PROJ_FP16)
    nc = _CACHE[key]

    in_maps = _prep_in_maps(x, Wq, bq, Wk, bk, Wv, bv, Wo, bo, proj_fp16=PROJ_FP16)
    res = run_bass_kernel_spmd(nc, in_maps, core_ids=list(range(NCORES)))
    _CACHE["last_result"] = res
    return np.concatenate([res.results[c]["out"] for c in range(NCORES)], axis=0)


# revision 22
# speedup vs baseline: 1.4411x; 1.4411x over previous
"""Trainium2 Bass kernel for nn_MultiHeadAttention_53815940219243.

Reference computation (single-head attention with full 1024-dim contraction):
    q = x @ Wq + bq; k = x @ Wk + bk; v = x @ Wv + bv        # [4096, 1024]
    scores = softmax(q @ k.T, axis=-1) / sqrt(64)            # [4096, 4096]
    z = scores @ v                                           # [4096, 1024]
    out = z @ Wo + bo                                        # [4096, 64]

Sharding: sequence dim of Q/scores/output split across 8 cores (512 rows each).
K and V are computed sharded (each core its own 512 rows) and exchanged with
two AllGather collectives overlapped with the other projections.

Dataflow runs in "transposed score space" so no on-device transposes are
needed (see ST/UT below). High-precision matmuls use hi/lo-split bf16
operands (3 passes: hh, hl, lh), giving ~2e-4 matmul error at bf16 speed;
E=exp(S) and v are single bf16 (~1e-3), the final Wo matmul is fp32.

Softmax is computed without max subtraction: scores are ~N(0, 10.7^2), so
|S| < ~60 and exp(S) stays inside fp32/bf16 range.
"""

import numpy as np

N = 4096
D = 1024
H = 64
NCORES = 8
NSH = N // NCORES  # 512 rows per core
P = 128
DT = D // P  # 8 partition tiles over the 1024 feature dim
JT = N // P  # 32 partition tiles over the full sequence
JSH = NSH // P  # 4 sequence tiles per core shard
IT = NSH // P  # 4 output row tiles per core

_CACHE = {}

# S-matmul mode: False = hi/lo-split bf16 (3 passes), True = single-pass fp16
S_FP16 = True
# projection mode: True = single-pass fp16 projections
PROJ_FP16 = True


def _build(upto="O", rep_a=1, rep_ag=1, rep_s=1, rep_u=1, rep_o=1, s_fp16=False, proj_fp16=False):
    import concourse.mybir as mybir
    import concourse.tile as tile
    from concourse import bacc
    from contextlib import ExitStack

    stages = {"A0": 0, "A": 0, "S": 1, "U": 2, "O": 3}
    lvl = stages[upto]
    no_ag = upto == "A0"

    f32 = mybir.dt.float32
    bf16 = mybir.dt.bfloat16
    fp16 = mybir.dt.float16
    sdt = fp16 if s_fp16 else bf16

    nc = bacc.Bacc("TRN2", target_bir_lowering=False, num_devices=NCORES)

    # ---- kernel I/O (per core) ----
    pdt = fp16 if proj_fp16 else bf16
    xh = nc.dram_tensor("xh", [D, NSH], pdt, kind="ExternalInput")
    xl = nc.dram_tensor("xl", [D, NSH], pdt, kind="ExternalInput")
    # weights pre-arranged on host: [t, p, dt*c] so each dout-tile load is
    # one contiguous-per-partition DMA
    wqh = nc.dram_tensor("wqh", [DT, P, D], pdt, kind="ExternalInput")
    wql = nc.dram_tensor("wql", [DT, P, D], pdt, kind="ExternalInput")
    wkh = nc.dram_tensor("wkh", [DT, P, D], pdt, kind="ExternalInput")
    wkl = nc.dram_tensor("wkl", [DT, P, D], pdt, kind="ExternalInput")
    wvh = nc.dram_tensor("wvh", [2, P, DT * 512], pdt, kind="ExternalInput")
    wvl = nc.dram_tensor("wvl", [2, P, DT * 512], pdt, kind="ExternalInput")
    wo = nc.dram_tensor("wo", [D, H], fp16, kind="ExternalInput")
    bq_r = nc.dram_tensor("bq_r", [1, D], pdt, kind="ExternalInput")
    bk_r = nc.dram_tensor("bk_r", [1, D], pdt, kind="ExternalInput")
    bv_r = nc.dram_tensor("bv_r", [1, D], pdt, kind="ExternalInput")
    bo_r = nc.dram_tensor("bo_r", [1, H], fp16, kind="ExternalInput")
    ones_row = nc.dram_tensor("ones_row", [1, NSH], pdt, kind="ExternalInput")
    ones_col = nc.dram_tensor("ones_col", [P, 1], bf16, kind="ExternalInput")
    eighth_row = nc.dram_tensor("eighth_row", [1, P], f32, kind="ExternalInput")
    ones32_row = nc.dram_tensor("ones32_row", [1, P], fp16, kind="ExternalInput")

    out = nc.dram_tensor("out", [NSH, H], f32, kind="ExternalOutput")

    # ---- internal DRAM for the collectives ----
    # layouts are partition-major so the post-gather streaming loads read
    # 16KB/8KB contiguous per partition line
    nkt = 1 if s_fp16 else 2
    ag_in_kt = nc.dram_tensor("ag_in_kt", [P, nkt, DT, NSH], sdt)
    ag_out_kt = nc.dram_tensor("ag_out_kt", [NCORES, P, nkt, DT, NSH], sdt, addr_space="Shared")
    ag_in_v = nc.dram_tensor("ag_in_v", [P, JSH, D], bf16)
    ag_out_v = nc.dram_tensor("ag_out_v", [NCORES, P, JSH, D], bf16, addr_space="Shared")

    with tile.TileContext(nc) as tc, ExitStack() as ctx:
        persist = ctx.enter_context(tc.tile_pool(name="persist", bufs=1))
        small = ctx.enter_context(tc.tile_pool(name="small", bufs=1))

        qth_sb = persist.tile([P, DT, NSH], sdt, tag="qth")
        qtl_sb = persist.tile([P, DT, NSH], sdt, tag="qtl")
        et_sb = persist.tile([P, JT, NSH], bf16, tag="et")        # 32KB/part
        vf_sb = persist.tile([P, NCORES, JSH, D], bf16, tag="vf")  # 64KB/part
        zt_sb = persist.tile([P, DT, NSH], fp16, tag="zt")        # 8KB/part
        sbc_sb = persist.tile([P, NSH], f32, tag="sbc")

        ones_row_sb = small.tile([1, NSH], pdt, tag="onesr")
        ones_col_sb = small.tile([P, 1], bf16, tag="onesc")
        eighth_sb = small.tile([1, P], f32, tag="eighth")
        ones32_sb = small.tile([1, P], fp16, tag="ones32")
        bq_sb = small.tile([1, D], pdt, tag="bq")
        bk_sb = small.tile([1, D], pdt, tag="bk")
        bv_sb = small.tile([1, D], pdt, tag="bv")
        bo_sb = small.tile([1, H], fp16, tag="bo")
        wo_sb = small.tile([P, DT, H], fp16, tag="wo")
        den_row = small.tile([1, NSH], f32, tag="denr")

        nc.sync.dma_start(out=ones_row_sb[:], in_=ones_row[:, :])
        nc.sync.dma_start(out=ones_col_sb[:], in_=ones_col[:, :])
        nc.sync.dma_start(out=eighth_sb[:], in_=eighth_row[:, :])
        nc.sync.dma_start(out=ones32_sb[:], in_=ones32_row[:, :])
        nc.sync.dma_start(out=bq_sb[:], in_=bq_r[:, :])
        nc.sync.dma_start(out=bk_sb[:], in_=bk_r[:, :])
        nc.sync.dma_start(out=bv_sb[:], in_=bv_r[:, :])
        nc.sync.dma_start(out=bo_sb[:], in_=bo_r[:, :])
        nc.sync.dma_start(out=wo_sb[:], in_=wo[:, :].rearrange("(t p) h -> p t h", p=P))

        # ---------------- phase A: projections ----------------
        with (
            tc.tile_pool(name="pa_x", bufs=1) as pax,
            tc.tile_pool(name="pa_sbuf", bufs=3) as pa,
            tc.tile_pool(name="pa_w", bufs=3) as paw,
            tc.tile_pool(name="pa_psum", bufs=4, space="PSUM") as pap,
        ):
            xh_sb = pax.tile([P, DT, NSH], pdt, tag="xh")
            xl_sb = pax.tile([P, DT, NSH], pdt, tag="xl")
            nc.sync.dma_start(out=xh_sb[:], in_=xh[:, :].rearrange("(t p) i -> p t i", p=P))
            nc.sync.dma_start(out=xl_sb[:], in_=xl[:, :].rearrange("(t p) i -> p t i", p=P))

            def proj_qk(w_h, w_l, b_sb, t):
                """One dout-tile of a hi/lo-split projection W^T @ xT + b."""
                wt_h = paw.tile([P, DT, P], pdt, tag="wqkh")
                nc.sync.dma_start(out=wt_h[:], in_=w_h[t].rearrange("p (dt c) -> p dt c", c=P))
                if not proj_fp16:
                    wt_l = paw.tile([P, DT, P], pdt, tag="wqkl")
                    nc.sync.dma_start(out=wt_l[:], in_=w_l[t].rearrange("p (dt c) -> p dt c", c=P))
                ps = pap.tile([P, NSH], mybir.dt.float32, tag="pa")
                for dt_i in range(DT):
                    nc.tensor.matmul(ps[:], wt_h[:, dt_i, :], xh_sb[:, dt_i, :],
                                     start=(dt_i == 0), stop=False)
                    if not proj_fp16:
                        nc.tensor.matmul(ps[:], wt_h[:, dt_i, :], xl_sb[:, dt_i, :],
                                         start=False, stop=False)
                        nc.tensor.matmul(ps[:], wt_l[:, dt_i, :], xh_sb[:, dt_i, :],
                                         start=False, stop=False)
                nc.tensor.matmul(ps[:], b_sb[0:1, t * P : (t + 1) * P],
                                 ones_row_sb[0:1, :], start=False, stop=True)
                return ps

            # K^T shard -> split hi/lo (or single fp16) -> ag_in_kt
            for _ra in range(rep_a):
              for t in range(DT):
                ps = proj_qk(wkh, wkl, bk_sb, t)
                kt_h = pa.tile([P, NSH], sdt, tag="kth")
                nc.vector.tensor_copy(out=kt_h[:], in_=ps[:])
                nc.sync.dma_start(out=ag_in_kt[:, 0, t, :], in_=kt_h[:])
                if not s_fp16:
                    kt_l = pa.tile([P, NSH], sdt, tag="ktl")
                    nc.vector.tensor_sub(out=kt_l[:], in0=ps[:], in1=kt_h[:])
                    nc.sync.dma_start(out=ag_in_kt[:, 1, t, :], in_=kt_l[:])

            if not no_ag:
                for _rg in range(rep_ag):
                    nc.gpsimd.collective_compute(
                        "AllGather", mybir.AluOpType.bypass,
                        replica_groups=[list(range(NCORES))],
                        ins=[ag_in_kt[:, :, :, :].opt()],
                        outs=[ag_out_kt[:, :, :, :, :].opt()],
                    )

            # v shard (natural layout, single bf16) -> ag_in_v
            for _ra in range(rep_a):
              for b in range(2):
                wv_h = paw.tile([P, DT, 512], pdt, tag="wvh")
                nc.sync.dma_start(out=wv_h[:], in_=wvh[b].rearrange("p (dt c) -> p dt c", c=512))
                if not proj_fp16:
                    wv_l = paw.tile([P, DT, 512], pdt, tag="wvl")
                    nc.sync.dma_start(out=wv_l[:], in_=wvl[b].rearrange("p (dt c) -> p dt c", c=512))
                for j in range(JSH):
                    ps = pap.tile([P, 512], mybir.dt.float32, tag="pa")
                    for dt_i in range(DT):
                        nc.tensor.matmul(ps[:], xh_sb[:, dt_i, j * P : (j + 1) * P],
                                         wv_h[:, dt_i, :], start=(dt_i == 0), stop=False)
                        if not proj_fp16:
                            nc.tensor.matmul(ps[:], xh_sb[:, dt_i, j * P : (j + 1) * P],
                                             wv_l[:, dt_i, :], start=False, stop=False)
                            nc.tensor.matmul(ps[:], xl_sb[:, dt_i, j * P : (j + 1) * P],
                                             wv_h[:, dt_i, :], start=False, stop=False)
                    nc.tensor.matmul(ps[:], ones_row_sb[0:1, 0:P],
                                     bv_sb[0:1, b * 512 : (b + 1) * 512],
                                     start=False, stop=True)
                    v_t = pa.tile([P, 512], bf16, tag="vsh")
                    nc.vector.tensor_copy(out=v_t[:], in_=ps[:])
                    nc.sync.dma_start(
                        out=ag_in_v[:, j, b * 512 : (b + 1) * 512], in_=v_t[:]
                    )

            if not no_ag:
                for _rg in range(rep_ag):
                    nc.gpsimd.collective_compute(
                        "AllGather", mybir.AluOpType.bypass,
                        replica_groups=[list(range(NCORES))],
                        ins=[ag_in_v[:, :, :].opt()],
                        outs=[ag_out_v[:, :, :, :].opt()],
                    )

            # Q^T (resident, split hi/lo or single fp16)
            for _ra in range(rep_a):
              for t in range(DT):
                ps = proj_qk(wqh, wql, bq_sb, t)
                nc.vector.tensor_copy(out=qth_sb[:, t, :], in_=ps[:])
                if not s_fp16:
                    nc.vector.tensor_sub(out=qtl_sb[:, t, :], in0=ps[:], in1=qth_sb[:, t, :])

        if lvl >= 1:
            # full v load (overlaps the S phase below)
            nc.sync.dma_start(
                out=vf_sb[:],
                in_=ag_out_v[:, :, :, :].rearrange("r p q d -> p r q d"),
            )

            # ------------- phase S: scores + exp + denominator -------------
            with (
                tc.tile_pool(name="ps_kt", bufs=4) as pskt,
                tc.tile_pool(name="ps_psum", bufs=4, space="PSUM") as psp,
                tc.tile_pool(name="ps_den", bufs=1, space="PSUM") as psd,
            ):
                den_ps = psd.tile([1, NSH], mybir.dt.float32, tag="den")
                for _rs in range(rep_s):
                  for r in range(NCORES):
                    kt_r = pskt.tile([P, nkt, DT, NSH], sdt, tag="ktr")
                    nc.sync.dma_start(out=kt_r[:], in_=ag_out_kt[r, :, :, :, :])
                    for jj in range(JSH):
                        jt = r * JSH + jj
                        ps = psp.tile([P, NSH], mybir.dt.float32, tag="st")
                        jsl = slice(jj * P, (jj + 1) * P)
                        for dt_i in range(DT):
                            last = dt_i == DT - 1
                            if s_fp16:
                                nc.tensor.matmul(ps[:], kt_r[:, 0, dt_i, jsl],
                                                 qth_sb[:, dt_i, :],
                                                 start=(dt_i == 0), stop=last)
                            else:
                                nc.tensor.matmul(ps[:], kt_r[:, 0, dt_i, jsl], qth_sb[:, dt_i, :],
                                                 start=(dt_i == 0), stop=False)
                                nc.tensor.matmul(ps[:], kt_r[:, 0, dt_i, jsl], qtl_sb[:, dt_i, :],
                                                 start=False, stop=False)
                                nc.tensor.matmul(ps[:], kt_r[:, 1, dt_i, jsl], qth_sb[:, dt_i, :],
                                                 start=False, stop=last)
                        nc.scalar.activation(out=et_sb[:, jt, :], in_=ps[:],
                                             func=mybir.ActivationFunctionType.Exp)
                        if jt > 0:
                            # one-tile lag: sum exp(S) of the previous tile so
                            # the PE never waits on the current tile's exp
                            nc.tensor.matmul(den_ps[:], ones_col_sb[:, 0:1],
                                             et_sb[:, jt - 1, :],
                                             start=(jt == 1), stop=False)
                  nc.tensor.matmul(den_ps[:], ones_col_sb[:, 0:1],
                                   et_sb[:, JT - 1, :], start=False, stop=True)

                # s = 1/(8*den) broadcast to 128 partitions (fp32 matmul)
                nc.vector.reciprocal(out=den_row[:], in_=den_ps[:])
                bc_ps = psd.tile([P, NSH], mybir.dt.float32, tag="bc")
                nc.tensor.matmul(bc_ps[:], eighth_sb[0:1, :], den_row[0:1, :],
                                 start=True, stop=True)
                nc.vector.tensor_copy(out=sbc_sb[:], in_=bc_ps[:])

        if lvl >= 2:
            # ---------------- phase U: z^T ----------------
            with tc.tile_pool(name="pu_psum", bufs=2, space="PSUM") as pup:
              for _ru in range(rep_u):
                for dt_i in range(DT):
                    ps = pup.tile([P, NSH], mybir.dt.float32, tag="ut")
                    dsl = slice(dt_i * P, (dt_i + 1) * P)
                    for jt in range(JT):
                        nc.tensor.matmul(ps[:], vf_sb[:, jt // JSH, jt % JSH, dsl],
                                         et_sb[:, jt, :],
                                         start=(jt == 0), stop=(jt == JT - 1))
                    nc.vector.tensor_mul(out=zt_sb[:, dt_i, :], in0=ps[:], in1=sbc_sb[:])

        if lvl >= 3:
            # ------------- phase O: out = z @ Wo + bo (fp32) -------------
            with (
                tc.tile_pool(name="po_sbuf", bufs=2) as po,
                tc.tile_pool(name="po_psum", bufs=2, space="PSUM") as pop,
            ):
              for _ro in range(rep_o):
                for it in range(IT):
                    ps = pop.tile([P, H], mybir.dt.float32, tag="o")
                    isl = slice(it * P, (it + 1) * P)
                    for dt_i in range(DT):
                        nc.tensor.matmul(ps[:], zt_sb[:, dt_i, isl], wo_sb[:, dt_i, :],
                                         start=(dt_i == 0), stop=False)
                    nc.tensor.matmul(ps[:], ones32_sb[0:1, :], bo_sb[0:1, :],
                                     start=False, stop=True)
                    o_t = po.tile([P, H], f32, tag="osb")
                    nc.vector.tensor_copy(out=o_t[:], in_=ps[:])
                    nc.sync.dma_start(out=out[isl, :], in_=o_t[:])
        else:
            with tc.tile_pool(name="dummy_o", bufs=1) as po:
                o_t = po.tile([P, H], f32, tag="osb")
                nc.vector.memset(o_t[:], 0.0)
                for it in range(IT):
                    nc.sync.dma_start(out=out[it * P : (it + 1) * P, :], in_=o_t[:])

    nc.finalize()
    return nc


def _split_bf16(a):
    import ml_dtypes
    a = np.ascontiguousarray(a, dtype=np.float32)
    hi = a.astype(ml_dtypes.bfloat16)
    lo = (a - hi.astype(np.float32)).astype(ml_dtypes.bfloat16)
    return hi, lo


def _prep_in_maps(x, Wq, bq, Wk, bk, Wv, bv, Wo, bo, proj_fp16=False):
    import ml_dtypes
    bf = np.float16 if proj_fp16 else ml_dtypes.bfloat16
    x = np.ascontiguousarray(x, dtype=np.float32)

    def split(a):
        if proj_fp16:
            h = np.ascontiguousarray(a, np.float32).astype(np.float16)
            return h, h
        return _split_bf16(a)

    def arr_qk(W):
        h, l = split(W)
        # [din, dout] -> [t, p, dt*c]: W'[t, p, dt, c] = W[dt*128+p, t*128+c]
        def re(a):
            return np.ascontiguousarray(
                a.reshape(DT, P, DT, P).transpose(2, 1, 0, 3).reshape(DT, P, D)
            )
        return re(h), re(l)

    def arr_v(W):
        h, l = split(W)
        def re(a):
            return np.ascontiguousarray(
                a.reshape(DT, P, 2, 512).transpose(2, 1, 0, 3).reshape(2, P, DT * 512)
            )
        return re(h), re(l)

    wqh, wql = arr_qk(Wq)
    wkh, wkl = arr_qk(Wk)
    wvh, wvl = arr_v(Wv)

    shared = {
        "wqh": wqh, "wql": wql, "wkh": wkh, "wkl": wkl, "wvh": wvh, "wvl": wvl,
        "wo": np.ascontiguousarray(Wo, dtype=np.float32).astype(np.float16),
        "bq_r": np.asarray(bq, np.float32).reshape(1, D).astype(bf),
        "bk_r": np.asarray(bk, np.float32).reshape(1, D).astype(bf),
        "bv_r": np.asarray(bv, np.float32).reshape(1, D).astype(bf),
        "bo_r": np.ascontiguousarray(bo, dtype=np.float32).reshape(1, H).astype(np.float16),
        "ones_row": np.ones((1, NSH), dtype=bf),
        "ones_col": np.ones((P, 1), dtype=ml_dtypes.bfloat16),
        "eighth_row": np.full((1, P), 0.125, dtype=np.float32),
        "ones32_row": np.ones((1, P), dtype=np.float16),
    }
    in_maps = []
    for c in range(NCORES):
        xcT = np.ascontiguousarray(x[c * NSH : (c + 1) * NSH, :].T)
        xch, xcl = split(xcT)
        m = dict(shared)
        m["xh"] = xch
        m["xl"] = xcl
        in_maps.append(m)
    return in_maps


def kernel(x, Wq, bq, Wk, bk, Wv, bv, Wo, bo):
    from concourse.bass_utils import run_bass_kernel_spmd

    key = ("nc", S_FP16, PROJ_FP16)
    if key not in _CACHE:
        _CACHE[key] = _build(s_fp16=S_FP16, proj_fp16=PROJ_FP16)
    nc = _CACHE[key]

    in_maps = _prep_in_maps(x, Wq, bq, Wk, bk, Wv, bv, Wo, bo, proj_fp16=PROJ_FP16)
    res = run_bass_kernel_spmd(nc, in_maps, core_ids=list(range(NCORES)))
    _CACHE["last_result"] = res
    return np.concatenate([res.results[c]["out"] for c in range(NCORES)], axis=0)
